# revision 35
# baseline (speedup 1.0000x reference)
"""Trainium2 Bass kernel for LinearSelfAttention (MobileViT-style).

Reference computation (per batch b, pixel p, patch n, channels c/o):
    qkv  = w_qkv @ x + b_qkv          # [B, 2C+1, P, N]
    q    = qkv[:, 0]                  # [B, P, N]
    key  = qkv[:, 1:1+C]
    val  = qkv[:, 1+C:]
    s    = softmax(q, axis=n)
    cv   = sum_n s * key              # [B, C, P]
    out  = w_out @ (relu(val) * cv[..., None]) + b_out

Strategy: data-parallel over B across 8 cores (2 batches each). Per core,
channels live on SBUF partitions; spatial (b, p, n) is the matmul moving
dim, processed in chunks of 512 (= 2 full p-rows, so softmax/reduce over
n stays chunk-local). All matmuls run in bf16 (full-rate PE + FWL weight
loads). Softmax skips the max-subtraction (|q| < ~5, exp is safe) and the
q bias cancels between numerator and Z. The key bias folds out of the
weighted sum (sum_n s == 1): cv = (W_k @ xs)/Z + b_k, xs = sum_n exp(q)*x.

PE work per chunk is 36 N=512 matmuls (4 q-broadcast + 16 value + 16
output projection) ~= 7.8us; the elementwise work (exp, xs-accumulate,
relu, *cv, out-bias) is spread across ACT/DVE/GpSimd so none exceeds
~55% occupancy and the PE never waits:
  - exp(q) is written in bf16 so the xs scalar_tensor_tensor runs in the
    DVE 2x packed mode; the *cv multiply is a DVE tensor_scalar (4x mode,
    per-partition pointer scalar) instead of a broadcast tensor_tensor.
  - relu(v+bv) and the mm2 bias-adds are split ACT / GpSimd tensor_scalar.
The tiny cv matmuls are batched over G=16 chunks and, crucially, issued
two chunks INTO the next group (not at the boundary) so the PE queue
never blocks on the DVE xs chain; output projection for chunk ch runs at
chunk ch+LAG against retained relu(v) tiles.
"""

import numpy as np
import ml_dtypes

import concourse.bass as bass
import concourse.mybir as mybir
import concourse.tile as tile
from concourse import bacc
from concourse.bass_utils import run_bass_kernel_spmd

B, C, P, N = 16, 512, 64, 256
NCORES = 8
BPC = B // NCORES          # batches per core
S = BPC * P * N            # spatial per core = 32768
SCH = 512                  # chunk = 2 p-rows
PCH = SCH // N             # p-rows per chunk = 2
NCH = S // SCH             # 64 chunks
CT = C // 128              # 4 channel tiles
G = 16                     # chunks per cv group
NG = NCH // G              # 4 groups
LAG = 19                   # chunks between a chunk's value pass and its mm2

F32 = mybir.dt.float32
BF = mybir.dt.bfloat16
F8 = mybir.dt.float8e4
AX = mybir.AxisListType
ALU = mybir.AluOpType
ACT = mybir.ActivationFunctionType

# Mixed-precision budget: fp8 DoubleRow halves the PE cost of whatever
# contraction it covers, at ~3.6%-rms element quantization noise.  The
# budget goes to the value projection (best savings per unit of error):
# for VK of the 4 value output tiles, contraction subtiles 2-3 run as one
# fp8 DR matmul (bf16 elsewhere).  The bf16 halves of those tiles carry a
# 2^17 weight scale so both halves accumulate on the same PSUM scale; the
# relu activation un-scales exactly.  Sim rel-err: VK=0: 2.8e-3, VK=3:
# 1.59e-2, VK=4: 1.85e-2 (gate 2e-2); QB_FP8 adds ~1.5e-2 in quadrature.
QB_FP8 = False
VK = 4
VSHIFT = 2.0 ** 17
XSC = 32.0        # x fp8 quantization scale (|x|max ~5.4 -> <240)
WSC = 4096.0      # weight fp8 quantization scale (|w|max ~0.044 -> <240)


def build():
    nc = bacc.Bacc("TRN2", target_bir_lowering=False, debug=False)

    x_d = nc.dram_tensor("x", [BPC, 128, CT, P, N], BF, kind="ExternalInput")
    w1v_d = nc.dram_tensor("w1v", [128, CT, C], BF, kind="ExternalInput")
    w1k_d = nc.dram_tensor("w1k", [128, CT, C], BF, kind="ExternalInput")
    QDT = F8 if QB_FP8 else BF
    w1q_d = nc.dram_tensor("w1q", [128, CT, 128], QDT, kind="ExternalInput")
    if QB_FP8 or VK:
        x8_d = nc.dram_tensor("x8", [BPC, 128, CT, P, N], F8,
                              kind="ExternalInput")
    if VK:
        w1v8_d = nc.dram_tensor("w1v8", [128, 2, VK * 128], F8,
                                kind="ExternalInput")
    w2_d = nc.dram_tensor("w2", [128, CT, C], BF, kind="ExternalInput")
    bk_d = nc.dram_tensor("bk", [128, CT], F32, kind="ExternalInput")
    bv_d = nc.dram_tensor("bv", [128, CT], F32, kind="ExternalInput")
    bo_d = nc.dram_tensor("bo", [128, CT], F32, kind="ExternalInput")
    y_d = nc.dram_tensor("y", [BPC, 128, CT, P, N], BF, kind="ExternalOutput")

    with tile.TileContext(nc) as tc:
        with (
            tc.tile_pool(name="wp", bufs=1) as wp,
            tc.tile_pool(name="xp", bufs=4) as xp,
            tc.tile_pool(name="x8p", bufs=4) as x8p,
            tc.tile_pool(name="eqbp", bufs=3) as eqbp,
            tc.tile_pool(name="zp", bufs=2) as zp,
            tc.tile_pool(name="rzp", bufs=2) as rzp,
            tc.tile_pool(name="xsp", bufs=2) as xsp,
            tc.tile_pool(name="xsbp", bufs=2) as xsbp,
            tc.tile_pool(name="cvp", bufs=2) as cvp,
            tc.tile_pool(name="scrp", bufs=4) as scrp,
            tc.tile_pool(name="rvp", bufs=LAG + 2) as rvp,
            tc.tile_pool(name="rsp", bufs=3) as rsp,
            tc.tile_pool(name="yop", bufs=3) as yop,
            tc.tile_pool(name="psqb", bufs=1, space="PSUM") as psqb,
            tc.tile_pool(name="psv", bufs=3, space="PSUM") as psv,
            tc.tile_pool(name="pscv", bufs=1, space="PSUM") as pscv,
            tc.tile_pool(name="psmm2", bufs=3, space="PSUM") as psmm2,
        ):
            # --- weights / constants, resident.  One DMA per tensor (the
            # per-issue queue cost ~630ns dominates these small loads).
            # sync queue carries only the per-chunk x DMAs so chunk 0's x
            # starts right after the preamble; weights go on the scalar
            # queue ordered by first use: w1q (chunk 0 qb), value weights
            # (chunk 0 value MMs), key weights last (first used ~chunk 18).
            w1q_a = wp.tile([128, CT, 128], QDT, name="w1q")
            nc.scalar.dma_start(out=w1q_a, in_=w1q_d[:, :, :])
            w1q_t = [w1q_a[:, ct] for ct in range(CT)]
            w1v_a = wp.tile([128, CT, C], BF, name="w1v")
            nc.scalar.dma_start(out=w1v_a, in_=w1v_d[:, :, :])
            w1v_t = [w1v_a[:, ct] for ct in range(CT)]
            if VK:
                w1v8_a = wp.tile([128, 2, VK * 128], F8, name="w1v8")
                nc.scalar.dma_start(out=w1v8_a, in_=w1v8_d[:, :, :])
            bv_t = wp.tile([128, CT], F32, name="bv_t")
            nc.scalar.dma_start(out=bv_t, in_=bv_d[:, :])
            w2_a = wp.tile([128, CT, C], BF, name="w2")
            nc.scalar.dma_start(out=w2_a, in_=w2_d[:, :, :])
            w2_t = [w2_a[:, ct] for ct in range(CT)]
            bo_t = wp.tile([128, CT], F32, name="bo_t")
            nc.scalar.dma_start(out=bo_t, in_=bo_d[:, :])
            bk_t = wp.tile([128, CT], F32, name="bk_t")
            nc.scalar.dma_start(out=bk_t, in_=bk_d[:, :])
            w1k_a = wp.tile([128, CT, C], BF, name="w1k")
            nc.scalar.dma_start(out=w1k_a, in_=w1k_d[:, :, :])
            w1k_t = [w1k_a[:, ct] for ct in range(CT)]

            rv_store = {}       # ch -> [4 rv tiles]
            rz_store = {}       # g -> [128, G, PCH] f32
            xs_store = {}       # g -> [128, G, CT, PCH] f32
            xsb_store = {}      # g -> [128, G, CT, PCH] bf16
            cvn_store = {}      # g -> cvn tile [128, CT, G, PCH] f32
            rs_pending = {}     # ch -> [4 rs tiles]

            def chunk_pos(ch):
                return ch // (P // PCH), (ch % (P // PCH)) * PCH

            def emit_rs(ch):
                # rs = rv * cvn via DVE tensor_scalar (bf16 4x mode)
                gx, jx = divmod(ch, G)
                cvn_g = cvn_store[gx]
                rv_list = rv_store.pop(ch)
                rs_list = []
                for i in range(CT):
                    rs = rsp.tile([128, PCH, N], BF, name=f"rs_{i}")
                    for p in range(PCH):
                        nc.vector.tensor_scalar_mul(
                            rs[:, p], rv_list[i][:, p],
                            cvn_g[:, i, jx, p:p + 1],
                        )
                    rs_list.append(rs)
                rs_pending[ch] = rs_list

            def emit_mm2(ch, split_y=False):
                bidx, p0 = chunk_pos(ch)
                rs_list = rs_pending.pop(ch)
                rs_f = [t.rearrange("c p n -> c (p n)") for t in rs_list]
                yo = yop.tile([128, CT, PCH, N], BF, name="yo")
                for o in range(CT):
                    mm2_ps = psmm2.tile([128, SCH], F32, name=f"mm2_{o}",
                                        tag="mm2")
                    for i in range(CT):
                        nc.tensor.matmul(
                            mm2_ps,
                            w2_t[i][:, o * 128:(o + 1) * 128],
                            rs_f[i],
                            start=(i == 0),
                            stop=(i == CT - 1),
                        )
                    mm2_3 = mm2_ps.rearrange("c (p n) -> c p n", p=PCH)
                    # in the flush ACT has no exp/relu work, so it takes
                    # all four drains and the DVE keeps ahead on rs
                    if o < 2 or split_y:
                        nc.scalar.activation(
                            yo[:, o], mm2_3, ACT.Identity,
                            bias=bo_t[:, o:o + 1],
                        )
                    else:
                        nc.vector.tensor_scalar_add(
                            yo[:, o], mm2_3, bo_t[:, o:o + 1]
                        )
                    # during the flush, overlap the y wire with the drains
                    if split_y and o == 1:
                        nc.sync.dma_start(
                            out=y_d[bidx, :, 0:2, p0:p0 + PCH, :],
                            in_=yo[:, 0:2],
                        )
                if split_y:
                    nc.sync.dma_start(
                        out=y_d[bidx, :, 2:4, p0:p0 + PCH, :],
                        in_=yo[:, 2:4],
                    )
                else:
                    nc.sync.dma_start(
                        out=y_d[bidx, :, :, p0:p0 + PCH, :], in_=yo,
                    )

            def emit_cv(gp):
                # batched cv for group gp: cv = W_k @ xs, then /Z + b_k
                cv_ps = pscv.tile([128, CT, G, PCH], F32, name="cv_ps")
                xsb = xsb_store.pop(gp)
                for i in range(CT):
                    for ct in range(CT):
                        nc.tensor.matmul(
                            cv_ps[:, i],
                            w1k_t[ct][:, i * 128:(i + 1) * 128],
                            xsb[:, :, ct, :],
                            start=(ct == 0), stop=(ct == CT - 1),
                        )
                cvn_g = cvp.tile([128, CT, G, PCH], F32, name="cvn_g")
                rz_g = rz_store.pop(gp)
                for i in range(CT):
                    nc.vector.tensor_mul(cvn_g[:, i], cv_ps[:, i], rz_g)
                    nc.vector.tensor_scalar_add(
                        cvn_g[:, i], cvn_g[:, i], bk_t[:, i:i + 1]
                    )
                cvn_store[gp] = cvn_g
                cvn_store.pop(gp - 2, None)

            for ch in range(NCH):
                g, j = divmod(ch, G)
                bidx, p0 = chunk_pos(ch)

                # --- x in: one DMA per chunk (queue issue cost dominates) ---
                xt = xp.tile([128, CT, PCH, N], BF, name="xt")
                if QB_FP8 or VK:
                    x8t = x8p.tile([128, CT, PCH, N], F8, name="x8t")
                    nc.sync.dma_start(
                        out=x8t, in_=x8_d[bidx, :, :, p0:p0 + PCH, :],
                    )
                    x8f = x8t.rearrange("c ct p n -> c ct (p n)")
                if ch < 2:
                    # startup: fan the first chunks' x across different
                    # engine queues -- parallel issue AND parallel wire --
                    # so chunk 0's x lands before the first value matmuls
                    qs = [nc.gpsimd, nc.gpsimd, nc.sync, nc.sync]
                    for h in range(CT):
                        qs[h].dma_start(
                            out=xt[:, h:h + 1],
                            in_=x_d[bidx, :, h:h + 1, p0:p0 + PCH, :],
                        )
                else:
                    nc.sync.dma_start(
                        out=xt, in_=x_d[bidx, :, :, p0:p0 + PCH, :],
                    )
                xf = [xt[:, ct] for ct in range(CT)]

                # --- group buffers ---
                if j == 0:
                    rz_store[g] = rzp.tile([128, G, PCH], F32, name="rz_g")
                    xs_store[g] = xsp.tile([128, G, CT, PCH], F32,
                                           name="xs_g")
                    xsb_store[g] = xsbp.tile([128, G, CT, PCH], BF,
                                             name="xs_bg")

                # --- old chunk's rs multiply: first in the DVE queue so
                # it is long done when the PE reaches that chunk's mm2 ---
                if ch >= LAG:
                    emit_rs(ch - LAG)

                # --- q, broadcast to 128 partitions via the rank-1
                # replicated weight trick: qb[c, s] = q[s] ---
                qb_ps = psqb.tile([128, SCH], F32, name="qb_ps", tag="qb")
                if QB_FP8:
                    for h in range(0, CT, 2):
                        nc.tensor.matmul(
                            qb_ps, w1q_a[:, h:h + 2, :], x8f[:, h:h + 2, :],
                            start=(h == 0), stop=(h == CT - 2),
                            perf_mode=mybir.MatmulPerfMode.DoubleRow,
                        )
                else:
                    for ct in range(CT):
                        nc.tensor.matmul(
                            qb_ps, w1q_t[ct], xf[ct],
                            start=(ct == 0), stop=(ct == CT - 1),
                        )
                qb3 = qb_ps.rearrange("c (p n) -> c p n", p=PCH)
                # exp in bf16: unlocks the DVE 2x packed mode for xs; the
                # activation scale un-does the fp8 quantization scales
                eqb = eqbp.tile([128, PCH, N], BF, name="eqb")
                z_sb = zp.tile([128, PCH], F32, name="z_sb")
                qsc = 1.0 / (XSC * WSC) if QB_FP8 else 1.0
                for p in range(PCH):
                    nc.scalar.activation(
                        eqb[:, p], qb3[:, p], ACT.Exp, scale=qsc,
                        accum_out=z_sb[:, p:p + 1],
                    )
                nc.vector.reciprocal(rz_store[g][:, j], z_sb)

                # --- xs = sum_n exp(q)*x  (weighted sum commutes with the
                # linear key projection: cv = W_k @ xs / Z + b_k) ---
                for ct in range(CT):
                    for p in range(PCH):
                        scr = scrp.tile([128, N], BF, name="scr", tag="scr")
                        nc.vector.scalar_tensor_tensor(
                            out=scr,
                            in0=eqb[:, p, :],
                            scalar=1.0,
                            in1=xt[:, ct, p, :],
                            op0=ALU.mult,
                            op1=ALU.mult,
                            accum_out=xs_store[g][:, j, ct, p:p + 1],
                        )
                if j == G - 1:
                    nc.vector.tensor_copy(xsb_store[g], xs_store[g])
                    xs_store.pop(g)

                # --- cv for the previous group, two chunks in: the PE
                # queue reaches these matmuls well after the DVE finished
                # that group's xs, so no boundary stall ---
                if j == 2 and g >= 1:
                    emit_cv(g - 1)

                # --- value tiles: for i < VK, subtiles 2-3 run as one fp8
                # DoubleRow matmul (weights pre-scaled so both halves share
                # the 2^17 PSUM scale, un-done by the relu's scale) ---
                rv_list = []
                for i in range(CT):
                    vp = psv.tile([128, SCH], F32, name=f"v_ps{i}", tag="v")
                    nct = 2 if i < VK else CT
                    for ct in range(nct):
                        nc.tensor.matmul(
                            vp,
                            w1v_t[ct][:, i * 128:(i + 1) * 128],
                            xf[ct],
                            start=(ct == 0),
                            stop=(i >= VK and ct == CT - 1),
                        )
                    if i < VK:
                        nc.tensor.matmul(
                            vp,
                            w1v8_a[:, :, i * 128:(i + 1) * 128],
                            x8f[:, 2:4, :],
                            start=False, stop=True,
                            perf_mode=mybir.MatmulPerfMode.DoubleRow,
                        )
                    rv = rvp.tile([128, PCH, N], BF, name=f"rv_{i}")
                    vp3 = vp.rearrange("c (p n) -> c p n", p=PCH)
                    nc.scalar.activation(
                        rv, vp3, ACT.Relu, bias=bv_t[:, i:i + 1],
                        scale=(1.0 / VSHIFT) if i < VK else 1.0,
                    )
                    rv_list.append(rv)
                rv_store[ch] = rv_list

                # --- an old chunk's output projection ---
                if ch >= LAG:
                    emit_mm2(ch - LAG)

                # --- prime the flush: two extra rs so the DVE stays two
                # tails ahead of the PE when the main loop ends ---
                if ch == NCH - 1:
                    emit_rs(NCH - LAG)
                    emit_rs(NCH - LAG + 1)

            # --- flush: the last group's cv goes right after the first
            # tail (its xs cast lands during chunk 63), so cvn[NG-1] and
            # the dependent rs stay well ahead of the PE's mm2 stream ---
            for k, ch in enumerate(range(NCH - LAG, NCH)):
                emit_mm2(ch, split_y=True)
                if k == 0:
                    emit_cv(NG - 1)
                if ch + 2 < NCH:
                    emit_rs(ch + 2)

    nc.compile()
    return nc


_NC = None


def _get_nc():
    global _NC
    if _NC is None:
        _NC = build()
    return _NC


def _prep_inputs(x, w_qkv, b_qkv, w_out, b_out):
    x = np.asarray(x, dtype=np.float32)
    w_qkv = np.asarray(w_qkv, dtype=np.float32)
    b_qkv = np.asarray(b_qkv, dtype=np.float32)
    w_out = np.asarray(w_out, dtype=np.float32)
    b_out = np.asarray(b_out, dtype=np.float32)

    def wlay(wT):
        # [C_in, O] -> [128, CT, O]: contraction tile ct on partitions
        return np.ascontiguousarray(
            wT.reshape(CT, 128, -1).transpose(1, 0, 2)
            .astype(ml_dtypes.bfloat16)
        )

    w1k = wlay(w_qkv[1:1 + C].T)
    w1vT = w_qkv[1 + C:].T.copy()            # [C_in, O]
    w1vT_bf = w1vT.copy()
    w1vT_bf[:, :VK * 128] *= VSHIFT
    w1v = wlay(w1vT_bf)
    if VK:
        # fp8 DoubleRow stationary: contraction subtiles 2,3 as the pair
        # axis, only the first VK output tiles
        w1v8 = np.ascontiguousarray(
            np.clip(w1vT[256:, :VK * 128] * WSC, -240, 240)
            .reshape(2, 128, VK * 128).transpose(1, 0, 2)
            .astype(ml_dtypes.float8_e4m3fn)
        )
    # q weight column replicated across 128 output partitions (rank-1
    # broadcast trick: (1 w_q^T)^T @ x = broadcast of q over partitions)
    w1q_full = np.repeat(w_qkv[0][:, None], 128, axis=1)
    if QB_FP8:
        w1q = np.ascontiguousarray(
            np.clip(w1q_full * WSC, -240, 240)
            .reshape(CT, 128, 128).transpose(1, 0, 2)
            .astype(ml_dtypes.float8_e4m3fn)
        )
    else:
        w1q = wlay(w1q_full)
    w2 = wlay(w_out.T)
    bk = np.ascontiguousarray(b_qkv[1:1 + C].reshape(CT, 128).T)
    bv = np.ascontiguousarray(b_qkv[1 + C:].reshape(CT, 128).T)
    bo = np.ascontiguousarray(b_out.reshape(CT, 128).T)

    # DRAM layout [BPC, 128, CT, P, N]: channel c = ct*128 + k lives at
    # [b, k, ct, p, n], so the per-chunk DMA slice is dim-order-matched
    # with the SBUF tile [128, CT, PCH, N] and needs a single transfer.
    xb = np.ascontiguousarray(
        x.reshape(NCORES, BPC, CT, 128, P, N)
        .transpose(0, 1, 3, 2, 4, 5)
        .astype(ml_dtypes.bfloat16)
    )
    shared = {"w1v": w1v, "w1k": w1k, "w1q": w1q, "w2": w2,
              "bk": bk, "bv": bv, "bo": bo}
    if VK:
        shared["w1v8"] = w1v8
    if QB_FP8 or VK:
        x8 = np.ascontiguousarray(
            np.clip(x.reshape(NCORES, BPC, CT, 128, P, N)
                    .transpose(0, 1, 3, 2, 4, 5) * XSC, -240, 240)
            .astype(ml_dtypes.float8_e4m3fn)
        )
        in_maps = [{"x": xb[i], "x8": x8[i], **shared}
                   for i in range(NCORES)]
    else:
        in_maps = [{"x": xb[i], **shared} for i in range(NCORES)]
    return in_maps


def run(in_maps, trace=False, **kwargs):
    nc = _get_nc()
    return run_bass_kernel_spmd(
        nc, in_maps, core_ids=list(range(NCORES)), trace=trace, **kwargs
    )


def _assemble(res):
    # y arrives as [BPC, 128, CT, P, N] per core; channel c = ct*128 + k
    y = np.stack([r["y"] for r in res.results], axis=0)
    return np.ascontiguousarray(
        y.astype(np.float32)
        .transpose(0, 1, 3, 2, 4, 5)
        .reshape(B, C, P, N)
    )


def kernel(x, w_qkv, b_qkv, w_out, b_out):
    in_maps = _prep_inputs(x, w_qkv, b_qkv, w_out, b_out)
    return _assemble(run(in_maps))
